# revision 20
# baseline (speedup 1.0000x reference)
"""Two-layer GCN (message passing) on 8 Trainium2 NeuronCores.

Strategy:
  - Shard dst nodes across 8 cores (12500 each, 98 blocks of 128 slots).
  - Edges partitioned by dst owner; per core, each (dst-block, src-group) pair
    gets a fixed 640-slot run (5 chunks of 128 edges, padded with null edges).
  - AllGathers are split into 4 piece-wise collectives; table rows are laid
    out piece-major so the gather window for group q is exactly the region
    AllGather q produces. Gathers start as soon as the first piece lands and
    the h2 AllGather overlaps the tail of layer 1.
  - Weighted one-hot matrices are built on the PE, not the DVE: per chunk a
    [24,128] compressed operand hlT (rows 0-15: w*onehot_hi, rows 16-23:
    onehot_lo) is multiplied by a constant 0/1 expansion matrix, giving
    x[e,d] = w*d_hi + d_lo in PSUM; oh = relu(x-1) equals w exactly on the
    matching column. The relu runs PSUM->SBUF on DVE/ACT (single-stream ops)
    so nothing grabs the shared DVE/GpSimd SBUF port and gather descriptor
    generation on the Q7s never stalls.
  - Layer aggregation: PE matmul accumulates msg^T @ onehot in PSUM
    (layer 1: [feat, dst]; layer 2 swaps operands: [dst, feat], which streams
    only 64 cols). relu -> @W2 -> h2 shard -> piecewise AllGather -> layer 2
    -> out (node-major). Stores are batched per superblock.
"""

import os
import sys

import numpy as np

for _p in ("/opt/trn_rl_repo", "/root/.axon_site/_ro/trn_rl_repo"):
    if os.path.isdir(_p) and _p not in sys.path:
        sys.path.append(_p)

import ml_dtypes  # noqa: E402

import concourse.bacc as bacc  # noqa: E402
import concourse.mybir as mybir  # noqa: E402
from concourse import library_config, tile  # noqa: E402
from concourse.bass_utils import run_bass_kernel_spmd  # noqa: E402

BF16 = ml_dtypes.bfloat16

# ---- problem constants (nn_BaselineGCN: N=100000, E=1600000, 128->128->64) ----
N_NODES = 100000
N_EDGES = 1600000
F_IN = 128
F_HID = 128
F_OUT = 64

NCORES = 8
NPC = N_NODES // NCORES          # 12500 dst nodes per core
BLK = 128                        # nodes per dst block
NBLK = (NPC + BLK - 1) // BLK    # 98 blocks per core
SLOTPC = NBLK * BLK              # 12544 node slots per core (44 dummies)
NG = 4                           # src groups = AllGather pieces
P1 = NPC // NG                   # 3125 rows per L1 shard piece
P2 = SLOTPC // NG                # 3136 rows per L2 shard piece
G1 = NCORES * P1                 # 25000 rows per L1 gather window
G2 = NCORES * P2                 # 25088 rows per L2 gather window
CPB_G = 5                        # chunks per (block, group) run
RUNSLOTS = CPB_G * BLK           # 640 edge slots per run
KBLK = CPB_G * NG                # 20 chunks per block
SBB = 7                          # blocks per superblock
NSB = NBLK // SBB                # 14 superblocks
CH_SB = SBB * KBLK               # 140 chunks per superblock
SLOT_SB = CH_SB * 128            # 17920 edge slots per superblock
NCHUNK = NBLK * KBLK             # 1960 chunks per core per layer
NSLOT = NCHUNK * 128             # 250880 edge slots per core per layer
IDXCOLS = NSLOT // 16            # idx16 tensor free dim
IDXCOLS_SB = SLOT_SB // 16       # 1120 per superblock
IDXCOLS_G = RUNSLOTS * SBB // 16  # 280 idx cols per (superblock, group) call
NIDX_CALL = RUNSLOTS * SBB       # 4480 indices per gather call
KHL = 24                         # hlT rows: 16 hi + 8 lo

_CACHE: dict = {}


def _wrap_idx16(v: np.ndarray) -> np.ndarray:
    """Pack indices for dma_gather: index i -> [i%16, i//16], replicated
    across the 8 groups of 16 partitions."""
    block = v.astype(np.int16).reshape(-1, 16).T  # [16, n/16]
    return np.tile(block, (8, 1))                 # [128, n/16]


def _layer_prep(es, ed_loc, w, piece, window_of):
    """Slot layout for one layer. es: src node ids, ed_loc: dst local ids,
    w: weights, piece: rows per shard piece. Returns idx16, hlT."""
    o = es // NPC
    r = es - o * NPC
    g = r // piece
    b = ed_loc // BLK
    loc = ed_loc % BLK
    run = b * NG + g
    counts = np.bincount(run, minlength=NBLK * NG)
    if counts.max() > RUNSLOTS:
        raise RuntimeError(f"run overflow {counts.max()} > {RUNSLOTS}")

    order = np.argsort(run, kind="stable")
    run_s = run[order]
    start_of_run = np.searchsorted(run_s, np.arange(NBLK * NG))
    pos = np.arange(len(es)) - start_of_run[run_s]
    bs, gs = b[order], g[order]
    run_base = (bs // SBB) * SLOT_SB + gs * (SBB * RUNSLOTS) + (bs % SBB) * RUNSLOTS
    slot = run_base + pos

    idx = np.zeros(NSLOT, np.int64)
    idx[slot] = o[order] * piece + (r[order] % piece)

    hl = np.zeros((NSLOT, KHL), np.float32)
    loc_s = loc[order]
    hl[slot, loc_s // 8] = w[order]
    hl[slot, 16 + (loc_s % 8)] = 1.0
    # chunk-major transpose: [KHL, NCHUNK*128]
    hlT = np.ascontiguousarray(
        hl.reshape(NCHUNK, 128, KHL).transpose(2, 0, 1).reshape(
            KHL, NSLOT)).astype(BF16)
    return _wrap_idx16(idx), hlT


def _prep_core(c: int, src: np.ndarray, dst: np.ndarray, ew: np.ndarray):
    m = (dst // NPC) == c
    es = src[m].astype(np.int64)
    ed = (dst[m] - c * NPC).astype(np.int64)
    w = ew[m].astype(np.float32)
    idx1, hlT1 = _layer_prep(es, ed, w, P1, G1)
    idx2, hlT2 = _layer_prep(es, ed, w, P2, G2)
    return idx1, hlT1, idx2, hlT2


def _bconst_np() -> np.ndarray:
    """Constant expansion matrix [KHL, 128]: rows 0-15 one-hot over d//8,
    rows 16-23 one-hot over d%8."""
    b = np.zeros((KHL, 128), np.float32)
    d = np.arange(128)
    b[d // 8, d] = 1.0
    b[16 + (d % 8), d] = 1.0
    return b.astype(BF16)


def _build_program():
    dbg_nsb = int(os.environ.get("KERNEL_DBG_NSB", str(NSB)))
    dbg_nogather = bool(int(os.environ.get("KERNEL_DBG_NOGATHER", "0")))
    dbg_nocoll = bool(int(os.environ.get("KERNEL_DBG_NOCOLL", "0")))
    nc = bacc.Bacc("TRN2", target_bir_lowering=False, debug=False,
                   num_devices=NCORES, num_swdge_queues=4,
                   dynamic_dma_scratch_size=24576)

    xT_d = nc.dram_tensor("xT", [F_IN, NPC], mybir.dt.bfloat16,
                          kind="ExternalInput")
    W1_d = nc.dram_tensor("W1b", [F_IN, F_HID], mybir.dt.bfloat16,
                          kind="ExternalInput")
    W2_d = nc.dram_tensor("W2b", [F_HID, F_OUT], mybir.dt.bfloat16,
                          kind="ExternalInput")
    idx1_d = nc.dram_tensor("idx1", [128, IDXCOLS], mybir.dt.int16,
                            kind="ExternalInput")
    idx2_d = nc.dram_tensor("idx2", [128, IDXCOLS], mybir.dt.int16,
                            kind="ExternalInput")
    hlT1_d = nc.dram_tensor("hlT1", [KHL, NSLOT], mybir.dt.bfloat16,
                            kind="ExternalInput")
    hlT2_d = nc.dram_tensor("hlT2", [KHL, NSLOT], mybir.dt.bfloat16,
                            kind="ExternalInput")
    bconst_d = nc.dram_tensor("bconst", [KHL, 128], mybir.dt.bfloat16,
                              kind="ExternalInput")
    out_d = nc.dram_tensor("outN", [SLOTPC, F_OUT], mybir.dt.float32,
                           kind="ExternalOutput")

    with tile.TileContext(nc) as tc:
        nc.gpsimd.load_library(library_config.mlp)
        with (
            tc.tile_pool(name="dram", bufs=1, space="DRAM") as dram,
            tc.tile_pool(name="const", bufs=1) as constp,
            tc.tile_pool(name="idxp", bufs=2) as idxp,
            tc.tile_pool(name="hlp", bufs=2) as hlp,
            tc.tile_pool(name="msgp", bufs=2) as msgp,
            tc.tile_pool(name="ohp", bufs=2) as ohp,
            tc.tile_pool(name="smallp", bufs=4) as smallp,
            tc.tile_pool(name="widep", bufs=2) as widep,
            tc.tile_pool(name="psagg", bufs=2, space="PSUM") as psagg,
            tc.tile_pool(name="psx", bufs=2, space="PSUM") as psx,
            tc.tile_pool(name="psgemm", bufs=2, space="PSUM") as psgemm,
        ):
            h_loc = dram.tile([NPC, F_HID], mybir.dt.bfloat16)
            h_piece = [dram.tile([G1, F_HID], mybir.dt.bfloat16,
                                 addr_space="Shared", tag=f"hp{q}",
                                 name=f"h_piece{q}")
                       for q in range(NG)]
            h2_loc = dram.tile([SLOTPC, 128], mybir.dt.bfloat16)
            h2_piece = [dram.tile([G2, 128], mybir.dt.bfloat16,
                                  addr_space="Shared", tag=f"h2p{q}",
                                  name=f"h2_piece{q}")
                        for q in range(NG)]

            w1_t = constp.tile([F_IN, F_HID], mybir.dt.bfloat16)
            nc.sync.dma_start(w1_t[:], W1_d[:])
            w2_t = constp.tile([F_HID, F_OUT], mybir.dt.bfloat16)
            nc.sync.dma_start(w2_t[:], W2_d[:])
            bc_t = constp.tile([KHL, 128], mybir.dt.bfloat16)
            nc.sync.dma_start(bc_t[:], bconst_d[:])
            neg1_t = constp.tile([128, 1], mybir.dt.float32)
            nc.vector.memset(neg1_t[:], -1.0)

            # ---- GEMM1: h_loc = (xT)^T @ W1, batched stores per load tile ----
            with tc.tile_pool(name="xtp", bufs=2) as xtp:
                XTW = 2048  # cols per load tile (16 blocks exactly)
                for t0 in range(0, NBLK, XTW // BLK):
                    ncols = min(XTW, NPC - t0 * BLK)
                    xt_t = xtp.tile([F_IN, XTW], mybir.dt.bfloat16)
                    nc.sync.dma_start(
                        xt_t[:, :ncols],
                        xT_d[:, t0 * BLK:t0 * BLK + ncols])
                    hw_t = xtp.tile([128, XTW], mybir.dt.bfloat16, tag="hw")
                    for tt in range(0, ncols, BLK):
                        nr = min(BLK, ncols - tt)
                        ps = psgemm.tile([128, F_HID], mybir.dt.float32,
                                         tag="gemm")
                        nc.tensor.matmul(
                            ps[:nr, :], xt_t[:, tt:tt + nr], w1_t[:],
                            start=True, stop=True,
                        )
                        nc.scalar.activation(
                            hw_t[:nr, tt:tt + F_HID], ps[:nr, :],
                            mybir.ActivationFunctionType.Copy)
                    ncf = (ncols // BLK) * BLK
                    if ncf:
                        nc.sync.dma_start(
                            h_loc[t0 * BLK:t0 * BLK + ncf, :]
                            .rearrange("(t p) f -> p t f", p=BLK),
                            hw_t[:, :ncf].rearrange("p (t f) -> p t f",
                                                    f=F_HID))
                    if ncols > ncf:
                        nr = ncols - ncf
                        nc.sync.dma_start(
                            h_loc[t0 * BLK + ncf:t0 * BLK + ncols, :],
                            hw_t[:nr, ncf:ncf + F_HID])

            # ---- piecewise AllGather of h table ----
            if dbg_nocoll:
                for q in range(NG):
                    nc.sync.dma_start(h_piece[q][:P1, :],
                                      h_loc[q * P1:(q + 1) * P1, :])
            else:
                for q in range(NG):
                    nc.gpsimd.collective_compute(
                        "AllGather",
                        mybir.AluOpType.bypass,
                        ins=[h_loc[q * P1:(q + 1) * P1, :].opt()],
                        outs=[h_piece[q].opt()],
                        replica_groups=[list(range(NCORES))],
                    )

            # h2 AllGather piece q triggers inside the L1 superblock loop.
            trig_sb = []
            for q in range(NG):
                last_blk = ((q + 1) * P2 + BLK - 1) // BLK
                sb_ready = (last_blk + SBB - 1) // SBB
                trig_sb.append(min(sb_ready + 1, NSB))

            # ---- layer loops ----
            for layer in (1, 2):
                idx_d = idx1_d if layer == 1 else idx2_d
                hlT_d = hlT1_d if layer == 1 else hlT2_d
                table = h_piece if layer == 1 else h2_piece
                fmm = F_HID if layer == 1 else F_OUT

                for sb in range(dbg_nsb):
                    idx_t = idxp.tile([128, IDXCOLS_SB], mybir.dt.int16)
                    nc.scalar.dma_start(
                        idx_t[:],
                        idx_d[:, sb * IDXCOLS_SB:(sb + 1) * IDXCOLS_SB])
                    msg_t = msgp.tile([128, CH_SB, 128], mybir.dt.bfloat16)
                    dbg_ngather = int(os.environ.get("KERNEL_DBG_NGATHER",
                                                     str(NG)))
                    if not dbg_nogather:
                        for g in range(dbg_ngather):
                            nc.gpsimd.dma_gather(
                                msg_t[:, g * (CH_SB // NG):(g + 1) * (CH_SB // NG), :],
                                table[g][:],
                                idx_t[:, g * IDXCOLS_G:(g + 1) * IDXCOLS_G],
                                NIDX_CALL, NIDX_CALL, 128,
                                single_packet=False, queue_num=g,
                            )
                    else:
                        nc.vector.memset(msg_t[:, 0, :], 0.0)

                    # ---- one-hot build on PE: x = hlT^T @ bconst ----
                    QCH = CH_SB // 4  # 35 chunks per quarter-load
                    hqs = []
                    for quar in range(4):
                        hq = hlp.tile([KHL, QCH * 128], mybir.dt.bfloat16,
                                      tag="hlT")
                        nc.scalar.dma_start(
                            hq[:],
                            hlT_d[:, (sb * CH_SB + quar * QCH) * 128:
                                  (sb * CH_SB + (quar + 1) * QCH) * 128])
                        hqs.append(hq)
                    oh_t = ohp.tile([128, CH_SB, 128], mybir.dt.bfloat16)
                    XG = 4  # chunks per psum-x group
                    for grp in range(CH_SB // XG):
                        xs = psx.tile([128, XG * 128], mybir.dt.float32,
                                      tag="psX")
                        for j in range(XG):
                            ch = grp * XG + j
                            cq, cr = ch // QCH, ch % QCH
                            nc.tensor.matmul(
                                xs[:, j * 128:(j + 1) * 128],
                                hqs[cq][:, cr * 128:(cr + 1) * 128],
                                bc_t[:],
                                start=(j == 0), stop=(j == XG - 1),
                                skip_group_check=True,
                            )
                        oh_flat = oh_t[:, grp * XG:(grp + 1) * XG, :] \
                            .rearrange("p c d -> p (c d)")
                        if grp % 3 == 0:
                            nc.scalar.activation(
                                oh_flat, xs[:],
                                mybir.ActivationFunctionType.Relu,
                                bias=neg1_t[:], scale=1.0)
                        else:
                            nc.vector.tensor_scalar(
                                oh_flat, xs[:], -1.0, 0.0,
                                mybir.AluOpType.add, mybir.AluOpType.max)

                    psA = psagg.tile([128, 512], mybir.dt.float32, tag="psA")

                    # g-major (chunks in gather order, so matmuls of group g
                    # start as soon as gather g lands). PSUM has_written clear
                    # on start=True is bank-wide: one start per bank per sb.
                    if layer == 1:
                        psB = psagg.tile([128, 512], mybir.dt.float32,
                                         tag="psB")

                        def agg_slice(bi, psA=psA, psB=psB):
                            pst = psA if bi < 4 else psB
                            j = bi if bi < 4 else bi - 4
                            return pst[:F_HID, j * 128:(j + 1) * 128]

                        for g in range(NG):
                            for bi in range(SBB):
                                for k in range(CPB_G):
                                    ch = g * (CH_SB // NG) + bi * CPB_G + k
                                    nc.tensor.matmul(
                                        agg_slice(bi),
                                        msg_t[:, ch, :], oh_t[:, ch, :],
                                        start=(g == 0 and k == 0
                                               and bi in (0, 4)),
                                        stop=(g == NG - 1 and k == CPB_G - 1
                                              and bi in (3, 6)),
                                        skip_group_check=True,
                                    )
                    else:
                        # swapped: oh stationary, msg moving (64 cols),
                        # out = [dst, feat] in one bank (7 x 64 cols).
                        def agg_slice(bi, psA=psA):
                            return psA[:, bi * F_OUT:(bi + 1) * F_OUT]

                        for g in range(NG):
                            for bi in range(SBB):
                                for k in range(CPB_G):
                                    ch = g * (CH_SB // NG) + bi * CPB_G + k
                                    nc.tensor.matmul(
                                        agg_slice(bi),
                                        oh_t[:, ch, :],
                                        msg_t[:, ch, :F_OUT],
                                        start=(g == 0 and k == 0 and bi == 0),
                                        stop=(g == NG - 1 and k == CPB_G - 1
                                              and bi == 6),
                                        skip_group_check=True,
                                    )

                    if layer == 1:
                        h2w_t = widep.tile([128, SBB * 128],
                                           mybir.dt.bfloat16, tag="h2w")
                        nc.vector.memset(h2w_t[:], 0.0)
                        for bi in range(SBB):
                            relu_t = smallp.tile([128, 128],
                                                 mybir.dt.bfloat16, tag="relu")
                            nc.scalar.activation(
                                relu_t[:], agg_slice(bi),
                                mybir.ActivationFunctionType.Relu)
                            h2ps = psgemm.tile([128, F_OUT], mybir.dt.float32,
                                               tag="gemm")
                            nc.tensor.matmul(h2ps[:], relu_t[:], w2_t[:],
                                             start=True, stop=True)
                            nc.scalar.activation(
                                h2w_t[:, bi * 128:bi * 128 + F_OUT], h2ps[:],
                                mybir.ActivationFunctionType.Copy)
                        b0 = sb * SBB
                        nc.sync.dma_start(
                            h2_loc[b0 * BLK:(b0 + SBB) * BLK, :]
                            .rearrange("(b p) f -> p b f", p=BLK),
                            h2w_t[:].rearrange("p (b f) -> p b f", f=128))
                        if not dbg_nocoll:
                            for q in range(NG):
                                if trig_sb[q] == sb + 1:
                                    nc.gpsimd.collective_compute(
                                        "AllGather",
                                        mybir.AluOpType.bypass,
                                        ins=[h2_loc[q * P2:(q + 1) * P2, :].opt()],
                                        outs=[h2_piece[q].opt()],
                                        replica_groups=[list(range(NCORES))],
                                    )
                    else:
                        ow_t = widep.tile([128, SBB * F_OUT],
                                          mybir.dt.float32, tag="ow")
                        nc.vector.tensor_scalar(
                            ow_t[:], psA[:, :SBB * F_OUT], 1.0, None,
                            mybir.AluOpType.mult)
                        b0 = sb * SBB
                        nc.sync.dma_start(
                            out_d[b0 * BLK:(b0 + SBB) * BLK, :]
                            .rearrange("(b p) f -> p b f", p=BLK),
                            ow_t[:].rearrange("p (b f) -> p b f", f=F_OUT))

                if layer == 1 and dbg_nocoll:
                    for q in range(NG):
                        nc.sync.dma_start(h2_piece[q][:P2, :],
                                          h2_loc[q * P2:(q + 1) * P2, :])

    nc.compile()
    return nc


def kernel(x, W1, W2, edge_weight, edge_index):
    x = np.asarray(x)
    W1 = np.asarray(W1)
    W2 = np.asarray(W2)
    ew = np.asarray(edge_weight)
    ei = np.asarray(edge_index)
    src, dst = ei[0].astype(np.int64), ei[1].astype(np.int64)

    if "nc" not in _CACHE:
        _CACHE["nc"] = _build_program()
    nc = _CACHE["nc"]

    w1b = W1.astype(BF16)
    w2b = W2.astype(BF16)
    bconst = _bconst_np()

    in_maps = []
    for c in range(NCORES):
        idx1, hlT1, idx2, hlT2 = _prep_core(c, src, dst, ew)
        in_maps.append({
            "xT": np.ascontiguousarray(
                x[c * NPC:(c + 1) * NPC].T).astype(BF16),
            "W1b": w1b,
            "W2b": w2b,
            "idx1": idx1,
            "idx2": idx2,
            "hlT1": hlT1,
            "hlT2": hlT2,
            "bconst": bconst,
        })

    trace = bool(int(os.environ.get("KERNEL_TRACE", "0")))
    res = run_bass_kernel_spmd(nc, in_maps, core_ids=list(range(NCORES)),
                               trace=trace)
    _CACHE["last_result"] = res

    out = np.empty((N_NODES, F_OUT), np.float32)
    for c in range(NCORES):
        out[c * NPC:(c + 1) * NPC] = res.results[c]["outN"][:NPC]
    return out


# revision 21
# speedup vs baseline: 1.2399x; 1.2399x over previous
"""Two-layer GCN (message passing) on 8 Trainium2 NeuronCores.

Strategy:
  - Shard dst nodes across 8 cores (12500 each, 98 blocks of 128 slots).
  - Edges partitioned by dst owner; per core, each (dst-block, src-group) pair
    gets a fixed 640-slot run (5 chunks of 128 edges, padded with null edges).
  - AllGathers are split into 4 piece-wise collectives; table rows are laid
    out piece-major so the gather window for group q is exactly the region
    AllGather q produces. Gathers start as soon as the first piece lands and
    the h2 AllGather overlaps the tail of layer 1.
  - Weighted one-hot matrices are built on the PE, not the DVE: per chunk a
    [24,128] compressed operand hlT (rows 0-15: w*onehot_hi, rows 16-23:
    onehot_lo) is multiplied by a constant 0/1 expansion matrix, giving
    x[e,d] = w*d_hi + d_lo in PSUM; oh = relu(x-1) equals w exactly on the
    matching column. The relu runs PSUM->SBUF on DVE/ACT (single-stream ops)
    so nothing grabs the shared DVE/GpSimd SBUF port and gather descriptor
    generation on the Q7s never stalls.
  - Layer aggregation: PE matmul accumulates msg^T @ onehot in PSUM
    (layer 1: [feat, dst]; layer 2 swaps operands: [dst, feat], which streams
    only 64 cols). relu -> @W2 -> h2 shard -> piecewise AllGather -> layer 2
    -> out (node-major). Stores are batched per superblock.
"""

import os
import sys

import numpy as np

for _p in ("/opt/trn_rl_repo", "/root/.axon_site/_ro/trn_rl_repo"):
    if os.path.isdir(_p) and _p not in sys.path:
        sys.path.append(_p)

import ml_dtypes  # noqa: E402

import concourse.bacc as bacc  # noqa: E402
import concourse.mybir as mybir  # noqa: E402
from concourse import library_config, tile  # noqa: E402
from concourse.bass_utils import run_bass_kernel_spmd  # noqa: E402

BF16 = ml_dtypes.bfloat16

# ---- problem constants (nn_BaselineGCN: N=100000, E=1600000, 128->128->64) ----
N_NODES = 100000
N_EDGES = 1600000
F_IN = 128
F_HID = 128
F_OUT = 64

NCORES = 8
NPC = N_NODES // NCORES          # 12500 dst nodes per core
BLK = 128                        # nodes per dst block
NBLK = (NPC + BLK - 1) // BLK    # 98 blocks per core
SLOTPC = NBLK * BLK              # 12544 node slots per core (44 dummies)
NG = 4                           # src groups = AllGather pieces
P1 = NPC // NG                   # 3125 rows per L1 shard piece
P2 = SLOTPC // NG                # 3136 rows per L2 shard piece
G1 = NCORES * P1                 # 25000 rows per L1 gather window
G2 = NCORES * P2                 # 25088 rows per L2 gather window
CPB_G = 5                        # chunks per (block, group) run
RUNSLOTS = CPB_G * BLK           # 640 edge slots per run
KBLK = CPB_G * NG                # 20 chunks per block
SBB = 7                          # blocks per superblock
NSB = NBLK // SBB                # 14 superblocks
CH_SB = SBB * KBLK               # 140 chunks per superblock
SLOT_SB = CH_SB * 128            # 17920 edge slots per superblock
NCHUNK = NBLK * KBLK             # 1960 chunks per core per layer
NSLOT = NCHUNK * 128             # 250880 edge slots per core per layer
IDXCOLS = NSLOT // 16            # idx16 tensor free dim
IDXCOLS_SB = SLOT_SB // 16       # 1120 per superblock
IDXCOLS_G = RUNSLOTS * SBB // 16  # 280 idx cols per (superblock, group) call
NIDX_CALL = RUNSLOTS * SBB       # 4480 indices per gather call
KHL = 24                         # hlT rows: 16 hi + 8 lo

_CACHE: dict = {}


def _wrap_idx16(v: np.ndarray) -> np.ndarray:
    """Pack indices for dma_gather: index i -> [i%16, i//16], replicated
    across the 8 groups of 16 partitions."""
    block = v.astype(np.int16).reshape(-1, 16).T  # [16, n/16]
    return np.tile(block, (8, 1))                 # [128, n/16]


def _layer_prep(es, ed_loc, w, piece, window_of):
    """Slot layout for one layer. es: src node ids, ed_loc: dst local ids,
    w: weights, piece: rows per shard piece. Returns idx16, hlT."""
    o = es // NPC
    r = es - o * NPC
    g = r // piece
    b = ed_loc // BLK
    loc = ed_loc % BLK
    run = b * NG + g
    counts = np.bincount(run, minlength=NBLK * NG)
    if counts.max() > RUNSLOTS:
        raise RuntimeError(f"run overflow {counts.max()} > {RUNSLOTS}")

    order = np.argsort(run, kind="stable")
    run_s = run[order]
    start_of_run = np.searchsorted(run_s, np.arange(NBLK * NG))
    pos = np.arange(len(es)) - start_of_run[run_s]
    bs, gs = b[order], g[order]
    run_base = (bs // SBB) * SLOT_SB + gs * (SBB * RUNSLOTS) + (bs % SBB) * RUNSLOTS
    slot = run_base + pos

    idx = np.zeros(NSLOT, np.int64)
    idx[slot] = o[order] * piece + (r[order] % piece)

    hl = np.zeros((NSLOT, KHL), np.float32)
    loc_s = loc[order]
    hl[slot, loc_s // 8] = w[order]
    hl[slot, 16 + (loc_s % 8)] = 1.0
    # chunk-major transpose: [KHL, NCHUNK*128]
    hlT = np.ascontiguousarray(
        hl.reshape(NCHUNK, 128, KHL).transpose(2, 0, 1).reshape(
            KHL, NSLOT)).astype(BF16)
    return _wrap_idx16(idx), hlT


def _prep_core(c: int, src: np.ndarray, dst: np.ndarray, ew: np.ndarray):
    m = (dst // NPC) == c
    es = src[m].astype(np.int64)
    ed = (dst[m] - c * NPC).astype(np.int64)
    w = ew[m].astype(np.float32)
    idx1, hlT1 = _layer_prep(es, ed, w, P1, G1)
    idx2, hlT2 = _layer_prep(es, ed, w, P2, G2)
    return idx1, hlT1, idx2, hlT2


def _bconst_np() -> np.ndarray:
    """Constant expansion matrix [KHL, 128]: rows 0-15 one-hot over d//8,
    rows 16-23 one-hot over d%8."""
    b = np.zeros((KHL, 128), np.float32)
    d = np.arange(128)
    b[d // 8, d] = 1.0
    b[16 + (d % 8), d] = 1.0
    return b.astype(BF16)


def _build_program():
    dbg_nsb = int(os.environ.get("KERNEL_DBG_NSB", str(NSB)))
    dbg_nogather = bool(int(os.environ.get("KERNEL_DBG_NOGATHER", "0")))
    dbg_nocoll = bool(int(os.environ.get("KERNEL_DBG_NOCOLL", "0")))
    nc = bacc.Bacc("TRN2", target_bir_lowering=False, debug=False,
                   num_devices=NCORES, num_swdge_queues=4,
                   dynamic_dma_scratch_size=24576)

    xT_d = nc.dram_tensor("xT", [F_IN, NPC], mybir.dt.bfloat16,
                          kind="ExternalInput")
    W1_d = nc.dram_tensor("W1b", [F_IN, F_HID], mybir.dt.bfloat16,
                          kind="ExternalInput")
    W2_d = nc.dram_tensor("W2b", [F_HID, F_OUT], mybir.dt.bfloat16,
                          kind="ExternalInput")
    idx1_d = nc.dram_tensor("idx1", [128, IDXCOLS], mybir.dt.int16,
                            kind="ExternalInput")
    idx2_d = nc.dram_tensor("idx2", [128, IDXCOLS], mybir.dt.int16,
                            kind="ExternalInput")
    hlT1_d = nc.dram_tensor("hlT1", [KHL, NSLOT], mybir.dt.bfloat16,
                            kind="ExternalInput")
    hlT2_d = nc.dram_tensor("hlT2", [KHL, NSLOT], mybir.dt.bfloat16,
                            kind="ExternalInput")
    bconst_d = nc.dram_tensor("bconst", [KHL, 128], mybir.dt.bfloat16,
                              kind="ExternalInput")
    out_d = nc.dram_tensor("outN", [SLOTPC, F_OUT], mybir.dt.float32,
                           kind="ExternalOutput")

    with tile.TileContext(nc) as tc:
        nc.gpsimd.load_library(library_config.mlp)
        with (
            tc.tile_pool(name="dram", bufs=1, space="DRAM") as dram,
            tc.tile_pool(name="const", bufs=1) as constp,
            tc.tile_pool(name="idxp", bufs=2) as idxp,
            tc.tile_pool(name="hlp", bufs=2) as hlp,
            tc.tile_pool(name="msgp", bufs=2) as msgp,
            tc.tile_pool(name="ohp", bufs=2) as ohp,
            tc.tile_pool(name="smallp", bufs=4) as smallp,
            tc.tile_pool(name="widep", bufs=2) as widep,
            tc.tile_pool(name="psagg", bufs=2, space="PSUM") as psagg,
            tc.tile_pool(name="psx", bufs=2, space="PSUM") as psx,
            tc.tile_pool(name="psgemm", bufs=2, space="PSUM") as psgemm,
        ):
            h_loc = dram.tile([NPC, F_HID], mybir.dt.bfloat16)
            h_piece = [dram.tile([G1, F_HID], mybir.dt.bfloat16,
                                 addr_space="Shared", tag=f"hp{q}",
                                 name=f"h_piece{q}")
                       for q in range(NG)]
            h2_loc = dram.tile([SLOTPC, 128], mybir.dt.bfloat16)
            h2_piece = [dram.tile([G2, 128], mybir.dt.bfloat16,
                                  addr_space="Shared", tag=f"h2p{q}",
                                  name=f"h2_piece{q}")
                        for q in range(NG)]

            w1_t = constp.tile([F_IN, F_HID], mybir.dt.bfloat16)
            nc.sync.dma_start(w1_t[:], W1_d[:])
            w2_t = constp.tile([F_HID, F_OUT], mybir.dt.bfloat16)
            nc.sync.dma_start(w2_t[:], W2_d[:])
            bc_t = constp.tile([KHL, 128], mybir.dt.bfloat16)
            nc.sync.dma_start(bc_t[:], bconst_d[:])
            neg1_t = constp.tile([128, 1], mybir.dt.float32)
            nc.vector.memset(neg1_t[:], -1.0)

            # ---- GEMM1: h_loc = (xT)^T @ W1, batched stores per load tile ----
            with tc.tile_pool(name="xtp", bufs=2) as xtp:
                XTW = 2048  # cols per load tile (16 blocks exactly)
                for t0 in range(0, NBLK, XTW // BLK):
                    ncols = min(XTW, NPC - t0 * BLK)
                    xt_t = xtp.tile([F_IN, XTW], mybir.dt.bfloat16)
                    nc.sync.dma_start(
                        xt_t[:, :ncols],
                        xT_d[:, t0 * BLK:t0 * BLK + ncols])
                    hw_t = xtp.tile([128, XTW], mybir.dt.bfloat16, tag="hw")
                    for tt in range(0, ncols, BLK):
                        nr = min(BLK, ncols - tt)
                        ps = psgemm.tile([128, F_HID], mybir.dt.float32,
                                         tag="gemm")
                        nc.tensor.matmul(
                            ps[:nr, :], xt_t[:, tt:tt + nr], w1_t[:],
                            start=True, stop=True,
                        )
                        nc.scalar.activation(
                            hw_t[:nr, tt:tt + F_HID], ps[:nr, :],
                            mybir.ActivationFunctionType.Copy)
                    ncf = (ncols // BLK) * BLK
                    if ncf:
                        nc.scalar.dma_start(
                            h_loc[t0 * BLK:t0 * BLK + ncf, :]
                            .rearrange("(t p) f -> p t f", p=BLK),
                            hw_t[:, :ncf].rearrange("p (t f) -> p t f",
                                                    f=F_HID))
                    if ncols > ncf:
                        nr = ncols - ncf
                        nc.scalar.dma_start(
                            h_loc[t0 * BLK + ncf:t0 * BLK + ncols, :],
                            hw_t[:nr, ncf:ncf + F_HID])

            # ---- piecewise AllGather of h table ----
            if dbg_nocoll:
                for q in range(NG):
                    nc.sync.dma_start(h_piece[q][:P1, :],
                                      h_loc[q * P1:(q + 1) * P1, :])
            else:
                for q in range(NG):
                    nc.gpsimd.collective_compute(
                        "AllGather",
                        mybir.AluOpType.bypass,
                        ins=[h_loc[q * P1:(q + 1) * P1, :].opt()],
                        outs=[h_piece[q].opt()],
                        replica_groups=[list(range(NCORES))],
                    )

            # h2 AllGather piece q triggers inside the L1 superblock loop.
            trig_sb = []
            for q in range(NG):
                last_blk = ((q + 1) * P2 + BLK - 1) // BLK
                sb_ready = (last_blk + SBB - 1) // SBB
                trig_sb.append(min(sb_ready + 1, NSB))

            # ---- layer loops ----
            for layer in (1, 2):
                idx_d = idx1_d if layer == 1 else idx2_d
                hlT_d = hlT1_d if layer == 1 else hlT2_d
                table = h_piece if layer == 1 else h2_piece
                fmm = F_HID if layer == 1 else F_OUT

                for sb in range(dbg_nsb):
                    idx_t = idxp.tile([128, IDXCOLS_SB], mybir.dt.int16)
                    nc.sync.dma_start(
                        idx_t[:],
                        idx_d[:, sb * IDXCOLS_SB:(sb + 1) * IDXCOLS_SB])
                    msg_t = msgp.tile([128, CH_SB, 128], mybir.dt.bfloat16)
                    dbg_ngather = int(os.environ.get("KERNEL_DBG_NGATHER",
                                                     str(NG)))
                    if not dbg_nogather:
                        for g in range(dbg_ngather):
                            nc.gpsimd.dma_gather(
                                msg_t[:, g * (CH_SB // NG):(g + 1) * (CH_SB // NG), :],
                                table[g][:],
                                idx_t[:, g * IDXCOLS_G:(g + 1) * IDXCOLS_G],
                                NIDX_CALL, NIDX_CALL, 128,
                                single_packet=False, queue_num=g,
                            )
                    else:
                        nc.vector.memset(msg_t[:, 0, :], 0.0)

                    # ---- one-hot build on PE: x = hlT^T @ bconst ----
                    QCH = CH_SB // 4  # 35 chunks per quarter-load
                    hqs = []
                    for quar in range(4):
                        hq = hlp.tile([KHL, QCH * 128], mybir.dt.bfloat16,
                                      tag="hlT")
                        nc.sync.dma_start(
                            hq[:],
                            hlT_d[:, (sb * CH_SB + quar * QCH) * 128:
                                  (sb * CH_SB + (quar + 1) * QCH) * 128])
                        hqs.append(hq)
                    oh_t = ohp.tile([128, CH_SB, 128], mybir.dt.bfloat16)
                    XG = 4  # chunks per psum-x group
                    for grp in range(CH_SB // XG):
                        xs = psx.tile([128, XG * 128], mybir.dt.float32,
                                      tag="psX")
                        for j in range(XG):
                            ch = grp * XG + j
                            cq, cr = ch // QCH, ch % QCH
                            nc.tensor.matmul(
                                xs[:, j * 128:(j + 1) * 128],
                                hqs[cq][:, cr * 128:(cr + 1) * 128],
                                bc_t[:],
                                start=(j == 0), stop=(j == XG - 1),
                                skip_group_check=True,
                            )
                        oh_flat = oh_t[:, grp * XG:(grp + 1) * XG, :] \
                            .rearrange("p c d -> p (c d)")
                        if grp % 3 == 0:
                            nc.scalar.activation(
                                oh_flat, xs[:],
                                mybir.ActivationFunctionType.Relu,
                                bias=neg1_t[:], scale=1.0)
                        else:
                            nc.vector.tensor_scalar(
                                oh_flat, xs[:], -1.0, 0.0,
                                mybir.AluOpType.add, mybir.AluOpType.max)

                    psA = psagg.tile([128, 512], mybir.dt.float32, tag="psA")

                    # g-major (chunks in gather order, so matmuls of group g
                    # start as soon as gather g lands). PSUM has_written clear
                    # on start=True is bank-wide: one start per bank per sb.
                    if layer == 1:
                        psB = psagg.tile([128, 512], mybir.dt.float32,
                                         tag="psB")

                        def agg_slice(bi, psA=psA, psB=psB):
                            pst = psA if bi < 4 else psB
                            j = bi if bi < 4 else bi - 4
                            return pst[:F_HID, j * 128:(j + 1) * 128]

                        for g in range(NG):
                            for bi in range(SBB):
                                for k in range(CPB_G):
                                    ch = g * (CH_SB // NG) + bi * CPB_G + k
                                    nc.tensor.matmul(
                                        agg_slice(bi),
                                        msg_t[:, ch, :], oh_t[:, ch, :],
                                        start=(g == 0 and k == 0
                                               and bi in (0, 4)),
                                        stop=(g == NG - 1 and k == CPB_G - 1
                                              and bi in (3, 6)),
                                        skip_group_check=True,
                                    )
                    else:
                        # swapped: oh stationary, msg moving (64 cols),
                        # out = [dst, feat] in one bank (7 x 64 cols).
                        def agg_slice(bi, psA=psA):
                            return psA[:, bi * F_OUT:(bi + 1) * F_OUT]

                        for g in range(NG):
                            for bi in range(SBB):
                                for k in range(CPB_G):
                                    ch = g * (CH_SB // NG) + bi * CPB_G + k
                                    nc.tensor.matmul(
                                        agg_slice(bi),
                                        oh_t[:, ch, :],
                                        msg_t[:, ch, :F_OUT],
                                        start=(g == 0 and k == 0 and bi == 0),
                                        stop=(g == NG - 1 and k == CPB_G - 1
                                              and bi == 6),
                                        skip_group_check=True,
                                    )

                    if layer == 1:
                        h2w_t = widep.tile([128, SBB * 128],
                                           mybir.dt.bfloat16, tag="h2w")
                        nc.vector.memset(h2w_t[:], 0.0)
                        for bi in range(SBB):
                            relu_t = smallp.tile([128, 128],
                                                 mybir.dt.bfloat16, tag="relu")
                            nc.scalar.activation(
                                relu_t[:], agg_slice(bi),
                                mybir.ActivationFunctionType.Relu)
                            h2ps = psgemm.tile([128, F_OUT], mybir.dt.float32,
                                               tag="gemm")
                            nc.tensor.matmul(h2ps[:], relu_t[:], w2_t[:],
                                             start=True, stop=True)
                            nc.scalar.activation(
                                h2w_t[:, bi * 128:bi * 128 + F_OUT], h2ps[:],
                                mybir.ActivationFunctionType.Copy)
                        b0 = sb * SBB
                        nc.scalar.dma_start(
                            h2_loc[b0 * BLK:(b0 + SBB) * BLK, :]
                            .rearrange("(b p) f -> p b f", p=BLK),
                            h2w_t[:].rearrange("p (b f) -> p b f", f=128))
                        if not dbg_nocoll:
                            for q in range(NG):
                                if trig_sb[q] == sb + 1:
                                    nc.gpsimd.collective_compute(
                                        "AllGather",
                                        mybir.AluOpType.bypass,
                                        ins=[h2_loc[q * P2:(q + 1) * P2, :].opt()],
                                        outs=[h2_piece[q].opt()],
                                        replica_groups=[list(range(NCORES))],
                                    )
                    else:
                        ow_t = widep.tile([128, SBB * F_OUT],
                                          mybir.dt.float32, tag="ow")
                        nc.vector.tensor_scalar(
                            ow_t[:], psA[:, :SBB * F_OUT], 1.0, None,
                            mybir.AluOpType.mult)
                        b0 = sb * SBB
                        nc.scalar.dma_start(
                            out_d[b0 * BLK:(b0 + SBB) * BLK, :]
                            .rearrange("(b p) f -> p b f", p=BLK),
                            ow_t[:].rearrange("p (b f) -> p b f", f=F_OUT))

                if layer == 1 and dbg_nocoll:
                    for q in range(NG):
                        nc.sync.dma_start(h2_piece[q][:P2, :],
                                          h2_loc[q * P2:(q + 1) * P2, :])

    nc.compile()
    return nc


def kernel(x, W1, W2, edge_weight, edge_index):
    x = np.asarray(x)
    W1 = np.asarray(W1)
    W2 = np.asarray(W2)
    ew = np.asarray(edge_weight)
    ei = np.asarray(edge_index)
    src, dst = ei[0].astype(np.int64), ei[1].astype(np.int64)

    if "nc" not in _CACHE:
        _CACHE["nc"] = _build_program()
    nc = _CACHE["nc"]

    w1b = W1.astype(BF16)
    w2b = W2.astype(BF16)
    bconst = _bconst_np()

    in_maps = []
    for c in range(NCORES):
        idx1, hlT1, idx2, hlT2 = _prep_core(c, src, dst, ew)
        in_maps.append({
            "xT": np.ascontiguousarray(
                x[c * NPC:(c + 1) * NPC].T).astype(BF16),
            "W1b": w1b,
            "W2b": w2b,
            "idx1": idx1,
            "idx2": idx2,
            "hlT1": hlT1,
            "hlT2": hlT2,
            "bconst": bconst,
        })

    trace = bool(int(os.environ.get("KERNEL_TRACE", "0")))
    res = run_bass_kernel_spmd(nc, in_maps, core_ids=list(range(NCORES)),
                               trace=trace)
    _CACHE["last_result"] = res

    out = np.empty((N_NODES, F_OUT), np.float32)
    for c in range(NCORES):
        out[c * NPC:(c + 1) * NPC] = res.results[c]["outN"][:NPC]
    return out
